# revision 21
# baseline (speedup 1.0000x reference)
"""Trainium2 Bass kernel for nn_Decoder_6055903887927 (gnn_message_passing).

Math (per irrep i, d_i in (1,3,5)):
  h = silu(silu(inv @ w1 + b1) @ w2 + b2)
  r2f = (h @ w3 + b3) * 1/sqrt(RBF)            # (A, RBF, F)
  sparse[t,f] += sum_{n,d,r} sph_i[n,t,d] * feat_i[n,f,d]
                             * rbf[n,t,r] * r2f[n,r,f]
  out[idx[t]] += sparse[t]                     # scatter-add into (N, F)

Strategy: CP-expansion of the einsum into one big matmul with contraction
axis K = (d, r, n) of size 9*16*128 = 18432:
  sparse^T[f, t] = sum_K W[K, f] * P[K, t]
  W[(d,r,n), f] = feat_d[n, f] * r2f[n, r, f]     (elementwise build, DVE)
  P[(d,r,n), t] = sph_d[n, t] * rbf_r[n, t]       (elementwise build, DVE)
Both builds have n on partitions -> matmul contracts partitions directly.
Builds and the matmul run in fp16 (cast host-side), accumulation fp32 PSUM.

Sharding: split the TARGET axis t (T=2048) across 8 cores (TL=256 each).
The einsum reduces over (n, d, r), not t, so each core's 256 output rows
are complete - NO inter-core communication is needed on device. Each core
scatter-adds its 256 rows directly into a private (N+64, F) fp32 output
(first occurrence of each grid row goes to that row; duplicate targets go
to spare rows N..N+63). The host sums the 8 private outputs and folds the
spare rows back in - the additive unshard of this sharding. This removes
the AllGather + entry barrier + gather/merge tail that dominated the
previous version (collective floor ~30us + 20us tail on this platform).

All DRAM inputs are pre-transposed on the host so every DMA is contiguous
per partition (no strided-descriptor DMAs).
"""

import sys

sys.path.insert(0, "/opt/trn_rl_repo")

import numpy as np

import concourse.bass as bass
import concourse.mybir as mybir
from concourse import bacc, tile
from concourse.masks import make_identity

A, T, NGRID, RBF, F = 128, 2048, 65536, 16, 128
DS = (1, 3, 5)
NDP = sum(DS)  # 9 d-planes
IRREP_OF_DG = [0, 1, 1, 1, 2, 2, 2, 2, 2]
NCORES = 8
TL = T // NCORES  # 256 targets per core
NSPARE = 64  # spare rows for duplicate targets within a core

F32 = mybir.dt.float32
BF16 = mybir.dt.bfloat16
F16 = mybir.dt.float16
I32 = mybir.dt.int32
MD_MAP = {"f32": F32, "bf16": BF16, "f16": F16}

MAIN_DTYPE = "f16"
_CACHE: dict = {}


def _build_program(b3_nonzero, main_dtype, repeats=1):
    md = MD_MAP[main_dtype]
    nc = bacc.Bacc(
        "TRN2", target_bir_lowering=False, debug=False, num_devices=NCORES
    )

    # Host layouts are already partition-major: every DMA below reads a
    # contiguous run per partition.
    invT_h = nc.dram_tensor("invT", [F, A], md, kind="ExternalInput")
    wmat_h = nc.dram_tensor("wmat", [F, 6 * F], md, kind="ExternalInput")
    bvec_h = nc.dram_tensor("bvec", [F, 6], F32, kind="ExternalInput")
    w3_h = nc.dram_tensor("w3", [F, 3 * RBF * F], md, kind="ExternalInput")
    if b3_nonzero:
        b3_h = nc.dram_tensor("b3", [1, 3 * RBF * F], md, kind="ExternalInput")
    featp_h = nc.dram_tensor("featp", [A, NDP * F], md, kind="ExternalInput")
    planes_h = nc.dram_tensor(
        "planes", [A, (NDP + RBF) * TL], md, kind="ExternalInput"
    )
    soffs_h = nc.dram_tensor("soffs", [128, 2], I32, kind="ExternalInput")
    out_h = nc.dram_tensor("out", [NGRID + NSPARE, F], F32, kind="ExternalOutput")
    outb_h = nc.dram_tensor("outb", [NGRID + NSPARE, F], F32, kind="ExternalOutput")

    with tile.TileContext(nc) as tc:
        with (
            tc.tile_pool(name="const", bufs=1) as const,
            tc.tile_pool(name="mlp", bufs=2) as mlp,
            tc.tile_pool(name="work", bufs=2) as work,
            tc.tile_pool(name="psacc", bufs=1, space="PSUM") as psacc,
            tc.tile_pool(name="pssm", bufs=1, space="PSUM") as pssm,
            tc.tile_pool(name="psbig", bufs=1, space="PSUM") as psbig,
            tc.tile_pool(name="pstr", bufs=2, space="PSUM") as pstr,
        ):
            # ---- warm the ACT Silu table before anything else so the
            # ---- 1.3us ACT_TABLE_LOAD doesn't land on the MLP path.
            warm = const.tile([1, 2], F32, tag="warm")
            nc.vector.memset(warm[:], 0.0)
            nc.scalar.activation(
                warm[:], warm[:], mybir.ActivationFunctionType.Silu
            )

            # ---- input loads. Both HWDGE queues (sync + scalar) carry the
            # ---- planes first — they gate the DVE build pipeline, which
            # ---- paces the whole kernel. Weights/featp follow. Nothing on
            # ---- the gpsimd SWDGE queue (it drains ~25x slower here).
            invT_t = const.tile([F, A], md, tag="invT")
            wmat_t = const.tile([F, 6 * F], md, tag="wmat")
            bvec_t = const.tile([F, 6], F32, tag="bvec")
            w3_t = const.tile([F, 3 * RBF * F], md, tag="w3")
            featp_t = const.tile([A, NDP * F], md, tag="featp")
            planes_t = const.tile([A, (NDP + RBF) * TL], md, tag="planes")
            soffs_t = const.tile([128, 2], I32, tag="soffs")

            HRB = RBF // 2
            QRB = RBF // 4
            s0 = RBF * TL
            # P0a needs (sph0, rbf[0:8]): sph0 + one rbf quarter on each
            # queue so the first build's inputs land as early as possible.
            nc.sync.dma_start(
                planes_t[:, s0:s0 + TL], planes_h[:, s0:s0 + TL]
            )
            nc.sync.dma_start(
                planes_t[:, : QRB * TL], planes_h[:, : QRB * TL]
            )
            nc.scalar.dma_start(
                planes_t[:, QRB * TL:HRB * TL],
                planes_h[:, QRB * TL:HRB * TL],
            )
            # sync: sph planes 1-4, wmat, w3c0, soffs
            nc.sync.dma_start(
                planes_t[:, s0 + TL:s0 + 5 * TL],
                planes_h[:, s0 + TL:s0 + 5 * TL],
            )
            nc.sync.dma_start(wmat_t[:], wmat_h[:])
            nc.sync.dma_start(w3_t[:, :RBF * F], w3_h[:, :RBF * F])
            nc.sync.dma_start(soffs_t[:], soffs_h[:])
            # scalar: rbf half 1, sph planes 5-8, invT, w3c1/c2, featp
            nc.scalar.dma_start(
                planes_t[:, HRB * TL:RBF * TL],
                planes_h[:, HRB * TL:RBF * TL],
            )
            nc.scalar.dma_start(
                planes_t[:, s0 + 5 * TL:], planes_h[:, s0 + 5 * TL:]
            )
            nc.scalar.dma_start(invT_t[:], invT_h[:])
            nc.scalar.dma_start(bvec_t[:], bvec_h[:])
            nc.scalar.dma_start(
                w3_t[:, RBF * F:2 * RBF * F], w3_h[:, RBF * F:2 * RBF * F]
            )
            nc.scalar.dma_start(
                w3_t[:, 2 * RBF * F:], w3_h[:, 2 * RBF * F:]
            )
            nc.scalar.dma_start(featp_t[:], featp_h[:])
            if b3_nonzero:
                b3_t = const.tile([1, 3 * RBF * F], md, tag="b3")
                nc.scalar.dma_start(b3_t[:], b3_h[:])
                ones_t = const.tile([1, A], md, tag="ones")
                nc.gpsimd.memset(ones_t[:], 1.0)
            ident = const.tile([128, 128], F32, tag="ident")
            make_identity(nc, ident[:])

            def sph(dg):
                return planes_t[:, (RBF + dg) * TL:(RBF + dg + 1) * TL]

            def rbf_all():
                return planes_t[:, :RBF * TL].rearrange(
                    "p (r t) -> p r t", r=RBF
                )

            for _rep in range(repeats):
                # ---- MLP (per irrep): r2f = silu(silu(inv@w1+b1)@w2+b2)@w3
                r2f_t = []
                for i in range(3):
                    h1p = pssm.tile([F, A], F32, tag="hsm", bufs=2, name="h1p")
                    nc.tensor.matmul(
                        h1p[:], wmat_t[:, i * F:(i + 1) * F], invT_t[:],
                        start=True, stop=True,
                    )
                    h1 = mlp.tile([F, A], md, tag="h1")
                    nc.scalar.activation(
                        h1[:], h1p[:], mybir.ActivationFunctionType.Silu,
                        bias=bvec_t[:, i:i + 1],
                    )
                    h2p = pssm.tile([F, A], F32, tag="hsm", bufs=2, name="h2p")
                    nc.tensor.matmul(
                        h2p[:], wmat_t[:, (3 + i) * F:(4 + i) * F], h1[:],
                        start=True, stop=True,
                    )
                    h2 = mlp.tile([F, A], md, tag="h2")
                    nc.scalar.activation(
                        h2[:], h2p[:], mybir.ActivationFunctionType.Silu,
                        bias=bvec_t[:, 3 + i:4 + i],
                    )
                    r2f = mlp.tile([A, RBF * F], md, tag=f"r2f_{i}")
                    for jh in range(2):
                        r2fp = psbig.tile([A, 1024], F32, tag="r2fp")
                        for j in range(2):
                            sl = slice(
                                i * RBF * F + jh * 1024 + j * 512,
                                i * RBF * F + jh * 1024 + (j + 1) * 512,
                            )
                            psl = slice(j * 512, (j + 1) * 512)
                            nc.tensor.matmul(
                                r2fp[:, psl], h2[:], w3_t[:, sl],
                                start=True, stop=not b3_nonzero,
                            )
                            if b3_nonzero:
                                nc.tensor.matmul(
                                    r2fp[:, psl], ones_t[:],
                                    b3_t[:, i * RBF * F + jh * 1024 + j * 512:
                                         i * RBF * F + jh * 1024 + (j + 1) * 512],
                                    start=False, stop=True,
                                )
                        nc.scalar.activation(
                            r2f[:, jh * 1024:(jh + 1) * 1024], r2fp[:],
                            mybir.ActivationFunctionType.Copy,
                        )
                    r2f_t.append(r2f)

                # ---- elementwise builds, split between DVE and GPSIMD so
                # ---- neither paces the whole phase. Schedule interleaves
                # ---- P/W on DVE so the PE contraction for plane dg can
                # ---- start as soon as (P_dg, W_dg) are both resident.
                pb_t = [
                    const.tile([A, RBF, TL], md, tag=f"pb{dg}", name=f"pb{dg}")
                    for dg in range(NDP)
                ]
                wb_t = [
                    const.tile([A, RBF, F], md, tag=f"wb{dg}", name=f"wb{dg}")
                    for dg in range(NDP)
                ]

                def build_p(eng, dg, rlo, rhi):
                    eng.tensor_mul(
                        pb_t[dg][:, rlo:rhi, :],
                        rbf_all()[:, rlo:rhi, :],
                        sph(dg).unsqueeze(1).broadcast_to(
                            [A, rhi - rlo, TL]
                        ),
                    )

                def build_w(eng, dg):
                    eng.tensor_mul(
                        wb_t[dg][:],
                        r2f_t[IRREP_OF_DG[dg]][:].rearrange(
                            "p (r f) -> p r f", r=RBF
                        ),
                        featp_t[:, dg * F:(dg + 1) * F]
                        .unsqueeze(1).broadcast_to([A, RBF, F]),
                    )

                # All builds on DVE (GPSIMD shares SBUF ports with DVE —
                # concurrent gpsimd TTs slow DVE ~2.7x, a net loss). Split
                # P0 into rbf halves to start before rbf fully loads.
                build_p(nc.vector, 0, 0, HRB)
                build_p(nc.vector, 0, HRB, RBF)
                build_w(nc.vector, 0)
                build_p(nc.vector, 1, 0, RBF)
                build_w(nc.vector, 1)
                build_p(nc.vector, 2, 0, RBF)
                build_w(nc.vector, 2)
                build_p(nc.vector, 3, 0, RBF)
                build_w(nc.vector, 3)
                build_p(nc.vector, 4, 0, RBF)
                build_w(nc.vector, 4)
                build_p(nc.vector, 5, 0, RBF)
                build_w(nc.vector, 5)
                build_p(nc.vector, 6, 0, RBF)
                build_w(nc.vector, 6)
                build_p(nc.vector, 7, 0, RBF)
                build_w(nc.vector, 7)
                build_p(nc.vector, 8, 0, RBF)
                build_w(nc.vector, 8)

                # ---- PE: single-pass contraction (144 matmuls, N=256) ----
                acc = psacc.tile([F, TL], F32, tag="acc")
                it = 0
                for dg in range(NDP):
                    for r in range(RBF):
                        nc.tensor.matmul(
                            acc[:],
                            wb_t[dg][:, r, :],
                            pb_t[dg][:, r, :],
                            start=(it == 0), stop=(it == NDP * RBF - 1),
                        )
                        it += 1
                accs = work.tile([F, TL], F32, tag="accs")
                nc.vector.tensor_copy(accs[:], acc[:])

                # ---- transpose to row-major fp32 and scatter the 256 rows
                # ---- straight into the private (N+64, F) outputs. Two
                # ---- separate output tensors so the two indirect DMAs
                # ---- don't serialize on a WAW dependency.
                for hh in range(2):
                    trp = pstr.tile([128, 128], F32, tag="trp", bufs=2)
                    nc.tensor.transpose(
                        trp[:], accs[:, hh * 128:(hh + 1) * 128], ident[:]
                    )
                    rl = work.tile([128, F], F32, tag=f"rl{hh}", name=f"rl{hh}")
                    if hh == 0:
                        nc.vector.tensor_copy(rl[:], trp[:])
                    else:
                        nc.scalar.activation(
                            rl[:], trp[:], mybir.ActivationFunctionType.Copy
                        )
                    nc.gpsimd.indirect_dma_start(
                        out=(out_h if hh == 0 else outb_h)[:],
                        out_offset=bass.IndirectOffsetOnAxis(
                            ap=soffs_t[:, hh:hh + 1], axis=0
                        ),
                        in_=rl[:],
                        in_offset=None,
                    )

    nc.compile()
    return nc


def _prep(inputs, main_dtype):
    """Host-side input prep -> (per-core in_maps, extras, b3_nonzero)."""
    if main_dtype == "f32":
        md = np.float32
    elif main_dtype == "f16":
        md = np.float16
    else:
        import ml_dtypes

        md = np.dtype(ml_dtypes.bfloat16)

    f0 = np.asarray(inputs["feat0"], np.float32)
    inv_rbf = np.float32(1.0 / np.sqrt(RBF))

    invT = np.ascontiguousarray(f0[:, :, 0].T).astype(md)
    w1 = np.asarray(inputs["mlp_w1"], np.float32)
    w2 = np.asarray(inputs["mlp_w2"], np.float32)
    # wmat host layout: [F_in, 6, F_out] -> contiguous per partition
    wmat = np.ascontiguousarray(
        np.concatenate([w1, w2], axis=0).transpose(1, 0, 2).reshape(F, 6 * F)
    ).astype(md)
    b1 = np.asarray(inputs["mlp_b1"], np.float32)
    b2 = np.asarray(inputs["mlp_b2"], np.float32)
    bvec = np.ascontiguousarray(
        np.concatenate([b1, b2], axis=0).T
    )  # [F, 6]
    w3f = np.asarray(inputs["mlp_w3"], np.float32) * inv_rbf  # (3, F, RBF*F)
    w3 = np.ascontiguousarray(
        w3f.transpose(1, 0, 2).reshape(F, 3 * RBF * F)
    ).astype(md)
    b3f = np.asarray(inputs["mlp_b3"], np.float32) * inv_rbf
    b3_nonzero = bool(np.any(b3f))
    b3 = b3f.reshape(1, 3 * RBF * F).astype(md)

    featp = np.ascontiguousarray(
        np.concatenate(
            [
                np.asarray(inputs[f"feat{i}"], np.float32).transpose(2, 0, 1)
                for i in range(3)
            ],
            axis=0,
        ).transpose(1, 0, 2).reshape(A, NDP * F)
    ).astype(md)  # (A, 9*F)
    sphp = np.concatenate(
        [
            np.asarray(inputs[f"sph{i}"], np.float32).transpose(2, 0, 1)
            for i in range(3)
        ],
        axis=0,
    )  # (9, A, T)
    rbfp = np.asarray(inputs["radial_basis_vals"], np.float32).transpose(
        2, 0, 1
    )  # (RBF, A, T)
    planes = np.concatenate([rbfp, sphp], axis=0).transpose(
        1, 0, 2
    )  # (A, 25, T)

    idx = np.asarray(inputs["truncated_idx"]).astype(np.int64)

    in_maps = []
    extras_all = []
    for c in range(NCORES):
        idx_c = idx[c * TL:(c + 1) * TL].tolist()
        soffs = np.zeros((128, 2), np.int32)
        seen = set()
        extras = []  # (half, spare_row, grid_row)
        for tl, v in enumerate(idx_c):
            if v in seen:
                dest = NGRID + len(extras)
                extras.append((tl // 128, dest, v))
            else:
                seen.add(v)
                dest = v
            # soffs[p, g] = dest for t = g*128 + p
            soffs[tl % 128, tl // 128] = dest
        assert len(extras) <= NSPARE
        extras_all.append(extras)
        ts = slice(c * TL, (c + 1) * TL)
        m = {
            "invT": invT,
            "wmat": wmat, "bvec": bvec, "w3": w3,
            "featp": featp,
            "planes": np.ascontiguousarray(planes[:, :, ts]).reshape(
                A, (NDP + RBF) * TL
            ).astype(md),
            "soffs": soffs,
        }
        if b3_nonzero:
            m["b3"] = b3
        in_maps.append(m)
    return in_maps, extras_all, b3_nonzero


def _get_runner(meta, b3_nonzero, main_dtype, repeats=1):
    key = (b3_nonzero, main_dtype, repeats)
    if key not in _CACHE:
        nc = _build_program(b3_nonzero, main_dtype, repeats)
        _CACHE[key] = nc
    return _CACHE[key]


def run_on_hw(in_maps, nc):
    from concourse import bass_utils

    res = bass_utils.run_bass_kernel_spmd(
        nc, in_maps, core_ids=list(range(NCORES))
    )
    return res.results


def kernel(**inputs) -> np.ndarray:
    in_maps, extras_all, b3_nonzero = _prep(inputs, MAIN_DTYPE)
    nc = _get_runner(extras_all, b3_nonzero, MAIN_DTYPE)
    results = run_on_hw(in_maps, nc)
    out = np.zeros((NGRID, F), np.float32)
    for c in range(NCORES):
        bufs = (results[c]["out"], results[c]["outb"])
        out += bufs[0][:NGRID]
        out += bufs[1][:NGRID]
        for g, srow, v in extras_all[c]:
            out[v] += bufs[g][srow]
    return out


# revision 22
# speedup vs baseline: 1.0320x; 1.0320x over previous
"""Trainium2 Bass kernel for nn_Decoder_6055903887927 (gnn_message_passing).

Math (per irrep i, d_i in (1,3,5)):
  h = silu(silu(inv @ w1 + b1) @ w2 + b2)
  r2f = (h @ w3 + b3) * 1/sqrt(RBF)            # (A, RBF, F)
  sparse[t,f] += sum_{n,d,r} sph_i[n,t,d] * feat_i[n,f,d]
                             * rbf[n,t,r] * r2f[n,r,f]
  out[idx[t]] += sparse[t]                     # scatter-add into (N, F)

Strategy: CP-expansion of the einsum into one big matmul with contraction
axis K = (d, r, n) of size 9*16*128 = 18432:
  sparse^T[f, t] = sum_K W[K, f] * P[K, t]
  W[(d,r,n), f] = feat_d[n, f] * r2f[n, r, f]     (elementwise build, DVE)
  P[(d,r,n), t] = sph_d[n, t] * rbf_r[n, t]       (elementwise build, DVE)
Both builds have n on partitions -> matmul contracts partitions directly.
Builds and the matmul run in fp16 (cast host-side), accumulation fp32 PSUM.

Sharding: split the TARGET axis t (T=2048) across 8 cores (TL=256 each).
The einsum reduces over (n, d, r), not t, so each core's 256 output rows
are complete - NO inter-core communication is needed on device. Each core
scatter-adds its 256 rows directly into a private (N+64, F) fp32 output
(first occurrence of each grid row goes to that row; duplicate targets go
to spare rows N..N+63). The host sums the 8 private outputs and folds the
spare rows back in - the additive unshard of this sharding. This removes
the AllGather + entry barrier + gather/merge tail that dominated the
previous version (collective floor ~30us + 20us tail on this platform).

All DRAM inputs are pre-transposed on the host so every DMA is contiguous
per partition (no strided-descriptor DMAs).
"""

import sys

sys.path.insert(0, "/opt/trn_rl_repo")

import numpy as np

import concourse.bass as bass
import concourse.mybir as mybir
from concourse import bacc, tile
from concourse.masks import make_identity

A, T, NGRID, RBF, F = 128, 2048, 65536, 16, 128
DS = (1, 3, 5)
NDP = sum(DS)  # 9 d-planes
IRREP_OF_DG = [0, 1, 1, 1, 2, 2, 2, 2, 2]
NCORES = 8
TL = T // NCORES  # 256 targets per core
NSPARE = 64  # spare rows for duplicate targets within a core

F32 = mybir.dt.float32
BF16 = mybir.dt.bfloat16
F16 = mybir.dt.float16
I32 = mybir.dt.int32
MD_MAP = {"f32": F32, "bf16": BF16, "f16": F16}

MAIN_DTYPE = "f16"
_CACHE: dict = {}


def _build_program(b3_nonzero, main_dtype, repeats=1):
    md = MD_MAP[main_dtype]
    nc = bacc.Bacc(
        "TRN2", target_bir_lowering=False, debug=False, num_devices=NCORES
    )

    # Host layouts are already partition-major: every DMA below reads a
    # contiguous run per partition.
    invT_h = nc.dram_tensor("invT", [F, A], md, kind="ExternalInput")
    wmat_h = nc.dram_tensor("wmat", [F, 6 * F], md, kind="ExternalInput")
    bvec_h = nc.dram_tensor("bvec", [F, 6], F32, kind="ExternalInput")
    w3_h = nc.dram_tensor("w3", [F, 3 * RBF * F], md, kind="ExternalInput")
    if b3_nonzero:
        b3_h = nc.dram_tensor("b3", [1, 3 * RBF * F], md, kind="ExternalInput")
    featp_h = nc.dram_tensor("featp", [A, NDP * F], md, kind="ExternalInput")
    planes_h = nc.dram_tensor(
        "planes", [A, (NDP + RBF) * TL], md, kind="ExternalInput"
    )
    soffs_h = nc.dram_tensor("soffs", [128, 2], I32, kind="ExternalInput")
    out_h = nc.dram_tensor("out", [NGRID + NSPARE, F], F32, kind="ExternalOutput")
    outb_h = nc.dram_tensor("outb", [NGRID + NSPARE, F], F32, kind="ExternalOutput")

    with tile.TileContext(nc) as tc:
        with (
            tc.tile_pool(name="const", bufs=1) as const,
            tc.tile_pool(name="mlp", bufs=2) as mlp,
            tc.tile_pool(name="work", bufs=2) as work,
            tc.tile_pool(name="psacc", bufs=1, space="PSUM") as psacc,
            tc.tile_pool(name="pssm", bufs=1, space="PSUM") as pssm,
            tc.tile_pool(name="psbig", bufs=1, space="PSUM") as psbig,
            tc.tile_pool(name="pstr", bufs=2, space="PSUM") as pstr,
        ):
            # ---- warm the ACT Silu table before anything else so the
            # ---- 1.3us ACT_TABLE_LOAD doesn't land on the MLP path.
            warm = const.tile([1, 2], F32, tag="warm")
            nc.vector.memset(warm[:], 0.0)
            nc.scalar.activation(
                warm[:], warm[:], mybir.ActivationFunctionType.Silu
            )

            # ---- input loads. Both HWDGE queues (sync + scalar) carry the
            # ---- planes first — they gate the DVE build pipeline, which
            # ---- paces the whole kernel. Weights/featp follow. Nothing on
            # ---- the gpsimd SWDGE queue (it drains ~25x slower here).
            invT_t = const.tile([F, A], md, tag="invT")
            wmat_t = const.tile([F, 6 * F], md, tag="wmat")
            bvec_t = const.tile([F, 6], F32, tag="bvec")
            w3_t = const.tile([F, 3 * RBF * F], md, tag="w3")
            featp_t = const.tile([A, NDP * F], md, tag="featp")
            planes_t = const.tile([A, (NDP + RBF) * TL], md, tag="planes")
            soffs_t = const.tile([128, 2], I32, tag="soffs")

            HRB = RBF // 2
            QRB = RBF // 4
            s0 = RBF * TL
            # P0a needs (sph0, rbf[0:8]): sph0 + one rbf quarter on each
            # queue so the first build's inputs land as early as possible.
            nc.sync.dma_start(
                planes_t[:, s0:s0 + TL], planes_h[:, s0:s0 + TL]
            )
            nc.sync.dma_start(
                planes_t[:, : QRB * TL], planes_h[:, : QRB * TL]
            )
            nc.scalar.dma_start(
                planes_t[:, QRB * TL:HRB * TL],
                planes_h[:, QRB * TL:HRB * TL],
            )
            # sync: sph plane 1, sph planes 2-4, wmat, w3c0, soffs
            nc.sync.dma_start(
                planes_t[:, s0 + TL:s0 + 2 * TL],
                planes_h[:, s0 + TL:s0 + 2 * TL],
            )
            nc.sync.dma_start(
                planes_t[:, s0 + 2 * TL:s0 + 5 * TL],
                planes_h[:, s0 + 2 * TL:s0 + 5 * TL],
            )
            nc.sync.dma_start(wmat_t[:], wmat_h[:])
            nc.sync.dma_start(w3_t[:, :RBF * F], w3_h[:, :RBF * F])
            nc.sync.dma_start(soffs_t[:], soffs_h[:])
            # scalar: rbf half 1, sph planes 5-8, invT, w3c1/c2, featp
            nc.scalar.dma_start(
                planes_t[:, HRB * TL:RBF * TL],
                planes_h[:, HRB * TL:RBF * TL],
            )
            nc.scalar.dma_start(
                planes_t[:, s0 + 5 * TL:], planes_h[:, s0 + 5 * TL:]
            )
            nc.scalar.dma_start(invT_t[:], invT_h[:])
            nc.scalar.dma_start(bvec_t[:], bvec_h[:])
            nc.scalar.dma_start(
                w3_t[:, RBF * F:2 * RBF * F], w3_h[:, RBF * F:2 * RBF * F]
            )
            nc.scalar.dma_start(
                w3_t[:, 2 * RBF * F:], w3_h[:, 2 * RBF * F:]
            )
            nc.scalar.dma_start(featp_t[:], featp_h[:])
            if b3_nonzero:
                b3_t = const.tile([1, 3 * RBF * F], md, tag="b3")
                nc.scalar.dma_start(b3_t[:], b3_h[:])
                ones_t = const.tile([1, A], md, tag="ones")
                nc.gpsimd.memset(ones_t[:], 1.0)
            ident = const.tile([128, 128], F32, tag="ident")
            make_identity(nc, ident[:])

            def sph(dg):
                return planes_t[:, (RBF + dg) * TL:(RBF + dg + 1) * TL]

            def rbf_all():
                return planes_t[:, :RBF * TL].rearrange(
                    "p (r t) -> p r t", r=RBF
                )

            for _rep in range(repeats):
                # ---- MLP (per irrep): r2f = silu(silu(inv@w1+b1)@w2+b2)@w3
                r2f_t = []
                for i in range(3):
                    h1p = pssm.tile([F, A], F32, tag="hsm", bufs=2, name="h1p")
                    nc.tensor.matmul(
                        h1p[:], wmat_t[:, i * F:(i + 1) * F], invT_t[:],
                        start=True, stop=True,
                    )
                    h1 = mlp.tile([F, A], md, tag="h1")
                    nc.scalar.activation(
                        h1[:], h1p[:], mybir.ActivationFunctionType.Silu,
                        bias=bvec_t[:, i:i + 1],
                    )
                    h2p = pssm.tile([F, A], F32, tag="hsm", bufs=2, name="h2p")
                    nc.tensor.matmul(
                        h2p[:], wmat_t[:, (3 + i) * F:(4 + i) * F], h1[:],
                        start=True, stop=True,
                    )
                    h2 = mlp.tile([F, A], md, tag="h2")
                    nc.scalar.activation(
                        h2[:], h2p[:], mybir.ActivationFunctionType.Silu,
                        bias=bvec_t[:, 3 + i:4 + i],
                    )
                    r2f = mlp.tile([A, RBF * F], md, tag=f"r2f_{i}")
                    for jh in range(2):
                        r2fp = psbig.tile([A, 1024], F32, tag="r2fp")
                        for j in range(2):
                            sl = slice(
                                i * RBF * F + jh * 1024 + j * 512,
                                i * RBF * F + jh * 1024 + (j + 1) * 512,
                            )
                            psl = slice(j * 512, (j + 1) * 512)
                            nc.tensor.matmul(
                                r2fp[:, psl], h2[:], w3_t[:, sl],
                                start=True, stop=not b3_nonzero,
                            )
                            if b3_nonzero:
                                nc.tensor.matmul(
                                    r2fp[:, psl], ones_t[:],
                                    b3_t[:, i * RBF * F + jh * 1024 + j * 512:
                                         i * RBF * F + jh * 1024 + (j + 1) * 512],
                                    start=False, stop=True,
                                )
                        nc.scalar.activation(
                            r2f[:, jh * 1024:(jh + 1) * 1024], r2fp[:],
                            mybir.ActivationFunctionType.Copy,
                        )
                    r2f_t.append(r2f)

                # ---- elementwise builds, split between DVE and GPSIMD so
                # ---- neither paces the whole phase. Schedule interleaves
                # ---- P/W on DVE so the PE contraction for plane dg can
                # ---- start as soon as (P_dg, W_dg) are both resident.
                pb_t = [
                    const.tile([A, RBF, TL], md, tag=f"pb{dg}", name=f"pb{dg}")
                    for dg in range(NDP)
                ]
                wb_t = [
                    const.tile([A, RBF, F], md, tag=f"wb{dg}", name=f"wb{dg}")
                    for dg in range(NDP)
                ]

                def build_p(eng, dg, rlo, rhi):
                    eng.tensor_mul(
                        pb_t[dg][:, rlo:rhi, :],
                        rbf_all()[:, rlo:rhi, :],
                        sph(dg).unsqueeze(1).broadcast_to(
                            [A, rhi - rlo, TL]
                        ),
                    )

                def build_w(eng, dg):
                    eng.tensor_mul(
                        wb_t[dg][:],
                        r2f_t[IRREP_OF_DG[dg]][:].rearrange(
                            "p (r f) -> p r f", r=RBF
                        ),
                        featp_t[:, dg * F:(dg + 1) * F]
                        .unsqueeze(1).broadcast_to([A, RBF, F]),
                    )

                # All builds on DVE (GPSIMD shares SBUF ports with DVE —
                # concurrent gpsimd TTs slow DVE ~2.7x, a net loss). Split
                # P0 into rbf halves to start before rbf fully loads.
                build_p(nc.vector, 0, 0, HRB)
                build_p(nc.vector, 0, HRB, RBF)
                build_w(nc.vector, 0)
                build_p(nc.vector, 1, 0, RBF)
                build_w(nc.vector, 1)
                build_p(nc.vector, 2, 0, RBF)
                build_w(nc.vector, 2)
                build_p(nc.vector, 3, 0, RBF)
                build_w(nc.vector, 3)
                build_p(nc.vector, 4, 0, RBF)
                build_w(nc.vector, 4)
                build_p(nc.vector, 5, 0, RBF)
                build_w(nc.vector, 5)
                build_p(nc.vector, 6, 0, RBF)
                build_w(nc.vector, 6)
                build_p(nc.vector, 7, 0, RBF)
                build_w(nc.vector, 7)
                build_p(nc.vector, 8, 0, RBF)
                build_w(nc.vector, 8)

                # ---- PE: single-pass contraction (144 matmuls, N=256) ----
                acc = psacc.tile([F, TL], F32, tag="acc")
                it = 0
                for dg in range(NDP):
                    for r in range(RBF):
                        nc.tensor.matmul(
                            acc[:],
                            wb_t[dg][:, r, :],
                            pb_t[dg][:, r, :],
                            start=(it == 0), stop=(it == NDP * RBF - 1),
                        )
                        it += 1
                accs = work.tile([F, TL], F32, tag="accs")
                nc.vector.tensor_copy(accs[:], acc[:])

                # ---- transpose to row-major fp32 and scatter the 256 rows
                # ---- straight into the private (N+64, F) outputs. Two
                # ---- separate output tensors so the two indirect DMAs
                # ---- don't serialize on a WAW dependency.
                for hh in range(2):
                    trp = pstr.tile([128, 128], F32, tag="trp", bufs=2)
                    nc.tensor.transpose(
                        trp[:], accs[:, hh * 128:(hh + 1) * 128], ident[:]
                    )
                    rl = work.tile([128, F], F32, tag=f"rl{hh}", name=f"rl{hh}")
                    if hh == 0:
                        nc.vector.tensor_copy(rl[:], trp[:])
                    else:
                        nc.scalar.activation(
                            rl[:], trp[:], mybir.ActivationFunctionType.Copy
                        )
                    nc.gpsimd.indirect_dma_start(
                        out=(out_h if hh == 0 else outb_h)[:],
                        out_offset=bass.IndirectOffsetOnAxis(
                            ap=soffs_t[:, hh:hh + 1], axis=0
                        ),
                        in_=rl[:],
                        in_offset=None,
                    )

    nc.compile()
    return nc


def _prep(inputs, main_dtype):
    """Host-side input prep -> (per-core in_maps, extras, b3_nonzero)."""
    if main_dtype == "f32":
        md = np.float32
    elif main_dtype == "f16":
        md = np.float16
    else:
        import ml_dtypes

        md = np.dtype(ml_dtypes.bfloat16)

    f0 = np.asarray(inputs["feat0"], np.float32)
    inv_rbf = np.float32(1.0 / np.sqrt(RBF))

    invT = np.ascontiguousarray(f0[:, :, 0].T).astype(md)
    w1 = np.asarray(inputs["mlp_w1"], np.float32)
    w2 = np.asarray(inputs["mlp_w2"], np.float32)
    # wmat host layout: [F_in, 6, F_out] -> contiguous per partition
    wmat = np.ascontiguousarray(
        np.concatenate([w1, w2], axis=0).transpose(1, 0, 2).reshape(F, 6 * F)
    ).astype(md)
    b1 = np.asarray(inputs["mlp_b1"], np.float32)
    b2 = np.asarray(inputs["mlp_b2"], np.float32)
    bvec = np.ascontiguousarray(
        np.concatenate([b1, b2], axis=0).T
    )  # [F, 6]
    w3f = np.asarray(inputs["mlp_w3"], np.float32) * inv_rbf  # (3, F, RBF*F)
    w3 = np.ascontiguousarray(
        w3f.transpose(1, 0, 2).reshape(F, 3 * RBF * F)
    ).astype(md)
    b3f = np.asarray(inputs["mlp_b3"], np.float32) * inv_rbf
    b3_nonzero = bool(np.any(b3f))
    b3 = b3f.reshape(1, 3 * RBF * F).astype(md)

    featp = np.ascontiguousarray(
        np.concatenate(
            [
                np.asarray(inputs[f"feat{i}"], np.float32).transpose(2, 0, 1)
                for i in range(3)
            ],
            axis=0,
        ).transpose(1, 0, 2).reshape(A, NDP * F)
    ).astype(md)  # (A, 9*F)
    sphp = np.concatenate(
        [
            np.asarray(inputs[f"sph{i}"], np.float32).transpose(2, 0, 1)
            for i in range(3)
        ],
        axis=0,
    )  # (9, A, T)
    rbfp = np.asarray(inputs["radial_basis_vals"], np.float32).transpose(
        2, 0, 1
    )  # (RBF, A, T)
    planes = np.concatenate([rbfp, sphp], axis=0).transpose(
        1, 0, 2
    )  # (A, 25, T)

    idx = np.asarray(inputs["truncated_idx"]).astype(np.int64)

    in_maps = []
    extras_all = []
    for c in range(NCORES):
        idx_c = idx[c * TL:(c + 1) * TL].tolist()
        soffs = np.zeros((128, 2), np.int32)
        seen = set()
        extras = []  # (half, spare_row, grid_row)
        for tl, v in enumerate(idx_c):
            if v in seen:
                dest = NGRID + len(extras)
                extras.append((tl // 128, dest, v))
            else:
                seen.add(v)
                dest = v
            # soffs[p, g] = dest for t = g*128 + p
            soffs[tl % 128, tl // 128] = dest
        assert len(extras) <= NSPARE
        extras_all.append(extras)
        ts = slice(c * TL, (c + 1) * TL)
        m = {
            "invT": invT,
            "wmat": wmat, "bvec": bvec, "w3": w3,
            "featp": featp,
            "planes": np.ascontiguousarray(planes[:, :, ts]).reshape(
                A, (NDP + RBF) * TL
            ).astype(md),
            "soffs": soffs,
        }
        if b3_nonzero:
            m["b3"] = b3
        in_maps.append(m)
    return in_maps, extras_all, b3_nonzero


def _get_runner(meta, b3_nonzero, main_dtype, repeats=1):
    key = (b3_nonzero, main_dtype, repeats)
    if key not in _CACHE:
        nc = _build_program(b3_nonzero, main_dtype, repeats)
        _CACHE[key] = nc
    return _CACHE[key]


def run_on_hw(in_maps, nc):
    from concourse import bass_utils

    res = bass_utils.run_bass_kernel_spmd(
        nc, in_maps, core_ids=list(range(NCORES))
    )
    return res.results


def kernel(**inputs) -> np.ndarray:
    in_maps, extras_all, b3_nonzero = _prep(inputs, MAIN_DTYPE)
    nc = _get_runner(extras_all, b3_nonzero, MAIN_DTYPE)
    results = run_on_hw(in_maps, nc)
    out = np.zeros((NGRID, F), np.float32)
    for c in range(NCORES):
        bufs = (results[c]["out"], results[c]["outb"])
        out += bufs[0][:NGRID]
        out += bufs[1][:NGRID]
        for g, srow, v in extras_all[c]:
            out[v] += bufs[g][srow]
    return out
